# revision 31
# baseline (speedup 1.0000x reference)
"""Trainium2 Bass kernel for RandomSparseNewMlp.

Math (reference):
    attn = (einsum('ds,td->st', fc1_w, fc2_w) + fc2_b) * sparse_mask   # [1024, 1024]
    out  = gelu_erf(einsum('bds,st->bdt', x, attn))                    # [64, 768, 1024]

Strategy (8 cores, SPMD, two NEFF dispatches, no collectives):
  NEFF A ("attn"): the [1024,1024] attn matrix is 2D-sharded over the 8
    cores (4-way along s, 2-way along t) — each core computes one
    [256, 512] slice from its fc1/fc2^T column slices, applies bias
    (folded into the contraction as an extra K-row: ones row in fc1,
    bias row in fc2^T, K padded 4096 -> 4224 = 33*128) and the sparse
    mask, and returns the masked fp16 slice.  The host concatenates the
    8 slices (pure layout, no arithmetic).  All NEFF A operands are
    host-pre-rearranged to partition-major [128, ...] layout so every
    DMA descriptor is a multi-KB contiguous run (512 B descriptors made
    the issuing engines descriptor-generation-bound).  fc2 streams on
    the sync DMA queue while fc1 + mask stream in parallel on the act
    queue, kd-chunked so the kd=0 matmuls start after ~1.5 us.
  NEFF B ("mlp"): data-parallel shard of x over batch; core c handles
    rows [c*6144, (c+1)*6144) of the flattened [49152, 1024] x, computes
    gelu(x @ attn) with the gathered attn as a replicated input.
    x^T is loaded in 4 r-blocks of [128, k, 1536] (3 KB contiguous
    descriptors per partition row, double-buffered).  The first 4
    row-tiles run k-outer so each arriving (attn_k, x_k) chunk unlocks
    8 matmuls — the PE stays ahead of the DMA stream during the
    cold-start window.  Outputs are stored fp16 in batches of 4
    row-tiles (the host upcasts); the last group stores per-row-tile to
    shorten the drain tail.

  PARTIAL_FP8: the 256 lowest-energy attn rows (by exact row energy of
    the fp16 attn, a pure host-side calibration scan) are permuted to
    the tail k-chunks and contracted as ONE DoubleRow fp8 matmul pair
    (e4m3, scale 1, K=256 at 2 rows/cycle) accumulating into the same
    PSUM group as the six fp16 k-chunks.  Cuts the per-row-tile matmul
    time ~11% for a verified-by-emulation absmax/scale of ~1.7e-2
    (gate 2e-2).  The s-permutation is applied on the host between the
    two NEFFs (slicing only; contraction order is matmul-invariant).
"""

import numpy as np
import ml_dtypes
from contextlib import ExitStack

import concourse.bass as bass  # noqa: F401  (engine registration side effects)
import concourse.mybir as mybir
import concourse.tile as tile
from concourse import bacc
from concourse import bass_utils

PARTIAL_FP8 = True

P = 128
B, D = 64, 768
IN_F, HID_F, OUT_F = 1024, 4096, 1024
N_CORES = 8
ROWS = B * D                    # 49152
ROWS_PC = ROWS // N_CORES       # 6144
KH = HID_F + P                  # 4224 = 33*128 (hidden + bias/ones row, padded)
KD = KH // P                    # 33
S_TILES = IN_F // P             # 8
K16 = S_TILES - 2 if PARTIAL_FP8 else S_TILES   # fp16 k-chunks
S16 = K16 * P                   # fp16 s-rows (768 with fp8 on)
NB = 512                        # matmul moving free dim / PSUM bank
RB = 1536                       # xt r-block width per DMA batch
NRB = ROWS_PC // RB             # 4
RT_PER_RB = RB // P             # 12
STG = 4                         # rt per output-store batch / k-outer start group
S_SH, T_SH = 4, 2               # attn sharding grid: 4 along s, 2 along t
S_SL = IN_F // S_SH             # 256 rows of attn per core
T_SL = OUT_F // T_SH            # 512 cols of attn per core

F32 = mybir.dt.float32
F16 = mybir.dt.float16
F8 = mybir.dt.float8e4
NP_F8 = ml_dtypes.float8_e4m3


def _trace_attn_kernel(tc, aslice, fc1s, fc2ts, masks):
    """Per-core attn slice (all partition-major [128, ...] layouts):

    fc1s  [128, 33*256] fp16 : fc1 (K-extended) cols, (p, kd, s) order
    fc2ts [128, 33*512] fp16 : fc2^T (K-extended) cols, (p, kd, t) order
    masks [128, 2*512]  f32  : mask slice, (p, j, t) order
    aslice[128, 2*512]  f16  : output slice, (p, j, t) order
    """
    nc = tc.nc
    fc1_r = fc1s.rearrange("p (k s) -> p k s", k=KD)    # [128, 33, 256]
    fc2_r = fc2ts.rearrange("p (k t) -> p k t", k=KD)   # [128, 33, 512]
    mask_r = masks.rearrange("p (j t) -> p j t", j=2)   # [128, 2, 512]
    out_r = aslice.rearrange("p (j t) -> p j t", j=2)

    with ExitStack() as ctx:
        spool = ctx.enter_context(tc.tile_pool(name="spool", bufs=1))
        ppool = ctx.enter_context(tc.tile_pool(name="ppool", bufs=2, space="PSUM"))
        f2_sb = spool.tile([P, KD, T_SL], F16)
        f1_sb = spool.tile([P, KD, S_SL], F16)
        mask_sb = spool.tile([P, 2, T_SL], F16)
        out_sb = spool.tile([P, 2, T_SL], F16)
        # Dual-queue weight streaming, kd-chunked for pacing, alternating
        # fc2 (the big stream) across BOTH queues in kd order so the
        # matmul stream is never starved by a single queue's issue rate;
        # fc1 chunks interleave (smaller, ahead of need), mask last.
        CH = 3
        q = [nc.sync, nc.scalar]
        # tiny first chunk so the kd=0 matmul starts as early as possible;
        # taper the last chunks to single kd so the final matmuls aren't
        # gated on one multi-kd block.
        TAPER = 30
        nc.sync.dma_start(f2_sb[:, 0:1, :], fc2_r[:, 0:1, :])
        nc.scalar.dma_start(f1_sb[:, 0:1, :], fc1_r[:, 0:1, :])
        chunks = list(range(1, TAPER, CH)) + list(range(TAPER, KD))
        for i, c in enumerate(chunks):
            n = min(CH, TAPER - c) if c < TAPER else 1
            q[i % 2].dma_start(f2_sb[:, c:c + n, :], fc2_r[:, c:c + n, :])
            q[(i + 1) % 2].dma_start(f1_sb[:, c:c + n, :], fc1_r[:, c:c + n, :])
        nc.scalar.dma_start(mask_sb, mask_r)
        psums = [ppool.tile([P, NB], F32, name="ap") for _ in range(2)]
        for kd in range(KD):
            for si in range(2):
                nc.tensor.matmul(
                    psums[si],
                    f1_sb[:, kd, si * P:(si + 1) * P],
                    f2_sb[:, kd, :],
                    start=(kd == 0),
                    stop=(kd == KD - 1),
                )
        for si in range(2):
            nc.vector.tensor_mul(out_sb[:, si, :], psums[si], mask_sb[:, si, :])
            nc.sync.dma_start(out_r[:, si:si + 1, :], out_sb[:, si:si + 1, :])


def _trace_mlp_kernel(tc, out, attn, xt, attn8=None, xt8=None):
    """out[6144,1024] (fp16) = gelu(xT^T @ attn) for this core's row shard.

    attn [768|1024, 1024] f16, xt [768|1024, 6144] f16; with PARTIAL_FP8
    attn8 [256, 1024] fp8e4 and xt8 [256, 6144] fp8e4 carry the two tail
    k-chunks, contracted via one DoubleRow matmul per PSUM bank.
    """
    nc = tc.nc
    gelu = mybir.ActivationFunctionType.Gelu
    dr = mybir.MatmulPerfMode.DoubleRow
    attn_r = attn.rearrange("(k p) t -> p k t", p=P)    # [128, K16, 1024]
    xt_r = xt.rearrange("(k p) r -> p k r", p=P)        # [128, K16, 6144]
    if PARTIAL_FP8:
        attn8_r = attn8.rearrange("(k p) t -> p k t", p=P)   # [128, 2, 1024]
        xt8_r = xt8.rearrange("(k p) r -> p k r", p=P)       # [128, 2, 6144]

    with ExitStack() as ctx:
        consts = ctx.enter_context(tc.tile_pool(name="consts", bufs=1))
        attn_sb = consts.tile([P, K16, OUT_F], F16)
        attn8_sb = (consts.tile([P, 2, OUT_F], F8, name="attn8_sb")
                    if PARTIAL_FP8 else None)
        xpool = ctx.enter_context(tc.tile_pool(name="xpool", bufs=3))
        x8pool = ctx.enter_context(tc.tile_pool(name="x8pool", bufs=3)) \
            if PARTIAL_FP8 else None
        opool = ctx.enter_context(tc.tile_pool(name="opool", bufs=4))
        mpool = ctx.enter_context(tc.tile_pool(name="main_psum", bufs=8, space="PSUM"))

        # Interleave attn k-chunks with ONLY the first store-group's x
        # columns (W0) so the k-outer matmuls become runnable k-by-k with
        # a minimal DMA prefix; the rest of r-block 0 follows batched.
        W0 = STG * P
        xt0 = xpool.tile([P, K16, RB], F16, name="xt")
        xt8_0 = (x8pool.tile([P, 2, RB], F8, name="xt8")
                 if PARTIAL_FP8 else None)
        # Prefix split across BOTH HWDGE queues: attn k-chunks on sync,
        # the matching x columns on act — two parallel trigger streams
        # during the DMA ramp window.
        if PARTIAL_FP8:
            nc.sync.dma_start(attn8_sb, attn8_r)
            nc.scalar.dma_start(xt8_0[:, :, 0:W0], xt8_r[:, :, 0:W0])
        for k in range(K16):
            nc.sync.dma_start(attn_sb[:, k:k + 1, :], attn_r[:, k:k + 1, :])
            nc.scalar.dma_start(xt0[:, k, 0:W0], xt_r[:, k, 0:W0])
        nc.sync.dma_start(xt0[:, :, W0:RB], xt_r[:, :, W0:RB])
        if PARTIAL_FP8:
            nc.scalar.dma_start(xt8_0[:, :, W0:RB], xt8_r[:, :, W0:RB])

        def mm_rt(xtile, x8tile, c0, pa, pb, split=False):
            # PSUM accumulation is commutative: the DoubleRow fp8 matmul
            # runs FIRST (carrying start=True) since its operands are
            # tiny and land earliest; the fp16 k-chunks close the group.
            if split:
                # final row-tile: close pa's group ~1.5us before pb's so
                # act+store of the pa half overlap the pb matmul chain
                for bank, off in ((pa, 0), (pb, NB)):
                    if PARTIAL_FP8:
                        x8s = x8tile[:, 0:2, c0:c0 + P]
                        nc.tensor.matmul(bank, x8s,
                                         attn8_sb[:, 0:2, off:off + NB],
                                         start=True, stop=False, perf_mode=dr)
                    for k in range(K16):
                        xs = xtile[:, k, c0:c0 + P]
                        first = (not PARTIAL_FP8) and (k == 0)
                        nc.tensor.matmul(bank, xs, attn_sb[:, k, off:off + NB],
                                         start=first, stop=(k == K16 - 1))
                return
            if PARTIAL_FP8:
                x8s = x8tile[:, 0:2, c0:c0 + P]
                nc.tensor.matmul(pa, x8s, attn8_sb[:, 0:2, 0:NB],
                                 start=True, stop=False, perf_mode=dr)
                nc.tensor.matmul(pb, x8s, attn8_sb[:, 0:2, NB:OUT_F],
                                 start=True, stop=False, perf_mode=dr)
            for k in range(K16):
                xs = xtile[:, k, c0:c0 + P]
                first = (not PARTIAL_FP8) and (k == 0)
                nc.tensor.matmul(pa, xs, attn_sb[:, k, 0:NB],
                                 start=first, stop=(k == K16 - 1))
                nc.tensor.matmul(pb, xs, attn_sb[:, k, NB:OUT_F],
                                 start=first, stop=(k == K16 - 1))

        def evict(ot, j, pa, pb, r0, batched):
            if not batched:
                # drain path: store each half right after its activation
                dst = out[r0:r0 + P, :].rearrange("(g p) t -> p g t", p=P)
                nc.scalar.activation(ot[:, j, 0:NB], pa, gelu)
                nc.sync.dma_start(dst[:, :, 0:NB], ot[:, j:j + 1, 0:NB])
                nc.scalar.activation(ot[:, j, NB:OUT_F], pb, gelu)
                nc.sync.dma_start(dst[:, :, NB:OUT_F], ot[:, j:j + 1, NB:OUT_F])
                return
            nc.scalar.activation(ot[:, j, 0:NB], pa, gelu)
            nc.scalar.activation(ot[:, j, NB:OUT_F], pb, gelu)
            if j == STG - 1:
                nc.sync.dma_start(
                    out[r0 - (STG - 1) * P:r0 + P, :].rearrange(
                        "(g p) t -> p g t", p=P),
                    ot,
                )

        for rb in range(NRB):
            if rb == 0:
                xtile, x8tile = xt0, (xt8_0 if PARTIAL_FP8 else None)
            else:
                xtile = xpool.tile([P, K16, RB], F16, name="xt")
                nc.sync.dma_start(xtile, xt_r[:, :, rb * RB:(rb + 1) * RB])
                if PARTIAL_FP8:
                    x8tile = x8pool.tile([P, 2, RB], F8, name="xt8")
                    nc.sync.dma_start(x8tile, xt8_r[:, :, rb * RB:(rb + 1) * RB])
                else:
                    x8tile = None
            if rb == 0:
                # k-outer start group: 4 rt x 2 banks per k-chunk, with
                # the DP fp8 matmuls first — their 0.33MB of operands is
                # the first data to land, so the PE has ~2us of work the
                # moment the DMA dead-start window ends.
                ps = [mpool.tile([P, NB], F32, name="mp") for _ in range(2 * STG)]
                if PARTIAL_FP8:
                    for j in range(STG):
                        x8s = x8tile[:, 0:2, j * P:(j + 1) * P]
                        nc.tensor.matmul(ps[2 * j], x8s, attn8_sb[:, 0:2, 0:NB],
                                         start=True, stop=False, perf_mode=dr)
                        nc.tensor.matmul(ps[2 * j + 1], x8s,
                                         attn8_sb[:, 0:2, NB:OUT_F],
                                         start=True, stop=False, perf_mode=dr)
                for k in range(K16):
                    for j in range(STG):
                        xs = xtile[:, k, j * P:(j + 1) * P]
                        first = (not PARTIAL_FP8) and (k == 0)
                        nc.tensor.matmul(ps[2 * j], xs, attn_sb[:, k, 0:NB],
                                         start=first, stop=(k == K16 - 1))
                        nc.tensor.matmul(ps[2 * j + 1], xs, attn_sb[:, k, NB:OUT_F],
                                         start=first, stop=(k == K16 - 1))
                ot = opool.tile([P, STG, OUT_F], F16, name="ot")
                for j in range(STG):
                    evict(ot, j, ps[2 * j], ps[2 * j + 1], (STG - 1) * P, True)
                rt_start = STG
            else:
                rt_start = 0
            for rt in range(rt_start, RT_PER_RB):
                c0 = rt * P
                last_group = (rb == NRB - 1 and rt >= RT_PER_RB - STG)
                if rt % STG == 0 and not last_group:
                    ot = opool.tile([P, STG, OUT_F], F16, name="ot")
                elif last_group:
                    ot = opool.tile([P, 1, OUT_F], F16, name="ot")
                pa = mpool.tile([P, NB], F32, name="mp")
                pb = mpool.tile([P, NB], F32, name="mp")
                final_rt = (rb == NRB - 1 and rt == RT_PER_RB - 1)
                mm_rt(xtile, x8tile, c0, pa, pb, split=final_rt)
                if last_group:
                    evict(ot, 0, pa, pb, rb * RB + c0, False)
                else:
                    evict(ot, rt % STG, pa, pb, rb * RB + c0, True)


_NC_CACHE = {}
LAST_RESULTS = None


def _build_attn():
    if "attn" in _NC_CACHE:
        return _NC_CACHE["attn"]
    nc = bacc.Bacc("TRN2", target_bir_lowering=False, debug=False,
                   num_devices=N_CORES)
    fc1s = nc.dram_tensor("fc1s", [P, KD * S_SL], F16, kind="ExternalInput").ap()
    fc2ts = nc.dram_tensor("fc2ts", [P, KD * T_SL], F16, kind="ExternalInput").ap()
    masks = nc.dram_tensor("masks", [P, 2 * T_SL], F16, kind="ExternalInput").ap()
    aslice = nc.dram_tensor("aslice", [P, 2 * T_SL], F16,
                            kind="ExternalOutput").ap()
    with tile.TileContext(nc) as tc:
        _trace_attn_kernel(tc, aslice, fc1s, fc2ts, masks)
    nc.compile()
    _NC_CACHE["attn"] = nc
    return nc


def _build_mlp():
    if "mlp" in _NC_CACHE:
        return _NC_CACHE["mlp"]
    nc = bacc.Bacc("TRN2", target_bir_lowering=False, debug=False,
                   num_devices=N_CORES)
    attn = nc.dram_tensor("attn", [S16, OUT_F], F16, kind="ExternalInput").ap()
    xt = nc.dram_tensor("xt", [S16, ROWS_PC], F16, kind="ExternalInput").ap()
    attn8 = xt8 = None
    if PARTIAL_FP8:
        attn8 = nc.dram_tensor("attn8", [2 * P, OUT_F], F8,
                               kind="ExternalInput").ap()
        xt8 = nc.dram_tensor("xt8", [2 * P, ROWS_PC], F8,
                             kind="ExternalInput").ap()
    out = nc.dram_tensor("out", [ROWS_PC, OUT_F], F16, kind="ExternalOutput").ap()
    with tile.TileContext(nc) as tc:
        _trace_mlp_kernel(tc, out, attn, xt, attn8, xt8)
    nc.compile()
    _NC_CACHE["mlp"] = nc
    return nc


def _run(nc, in_maps, **kwargs):
    return bass_utils.run_bass_kernel_spmd(
        nc, in_maps, core_ids=list(range(N_CORES)), **kwargs
    )


def _pmajor(a, inner):
    """[KD*128, inner] row-major -> [128, KD*inner] partition-major."""
    kd = a.shape[0] // P
    return np.ascontiguousarray(
        a.reshape(kd, P, inner).transpose(1, 0, 2).reshape(P, kd * inner)
    )


def kernel(x, fc1_w, fc2_w, fc2_b, sparse_mask, **run_kwargs):
    global LAST_RESULTS
    nc_a = _build_attn()
    nc_b = _build_mlp()

    # --- host prep: K-extended fp16 weight slices (layout only) ---
    fc1e = np.concatenate(
        [
            np.asarray(fc1_w, np.float32),
            np.ones((1, IN_F), np.float32),
            np.zeros((P - 1, IN_F), np.float32),
        ],
        axis=0,
    ).astype(np.float16)
    fc2te = np.concatenate(
        [
            np.asarray(fc2_w, np.float32).T,
            np.asarray(fc2_b, np.float32)[None, :],
            np.zeros((P - 1, OUT_F), np.float32),
        ],
        axis=0,
    ).astype(np.float16)
    mask = np.asarray(sparse_mask, np.float32)

    in_maps_a = []
    for c in range(N_CORES):
        si, tj = divmod(c, T_SH)
        in_maps_a.append({
            "fc1s": _pmajor(fc1e[:, si * S_SL:(si + 1) * S_SL], S_SL),
            "fc2ts": _pmajor(fc2te[:, tj * T_SL:(tj + 1) * T_SL], T_SL),
            "masks": _pmajor(
                np.ascontiguousarray(
                    mask[si * S_SL:(si + 1) * S_SL, tj * T_SL:(tj + 1) * T_SL]
                ).astype(np.float16), T_SL),
        })

    res_a = _run(nc_a, in_maps_a, **run_kwargs)

    # --- host gather of attn slices (pure concatenation / relayout) ---
    attn_full = np.empty((IN_F, OUT_F), np.float16)
    for c in range(N_CORES):
        si, tj = divmod(c, T_SH)
        attn_full[si * S_SL:(si + 1) * S_SL, tj * T_SL:(tj + 1) * T_SL] = (
            res_a.results[c]["aslice"].reshape(P, 2, T_SL)
            .transpose(1, 0, 2).reshape(S_SL, T_SL)
        )

    if PARTIAL_FP8:
        # Quantization calibration: permute the 256 lowest-energy attn
        # rows (and the matching x columns) into the fp8 tail chunks.
        energy = (attn_full.astype(np.float32) ** 2).sum(axis=1)
        perm = np.argsort(-energy)
        attn_p = attn_full[perm]
        attn16_h = np.ascontiguousarray(attn_p[:S16])
        attn8_h = np.ascontiguousarray(attn_p[S16:]).astype(NP_F8)
    else:
        perm = np.arange(IN_F)
        attn16_h = attn_full
        attn8_h = None

    x_flat = np.asarray(x, np.float32).reshape(ROWS, IN_F)
    in_maps_b = []
    for c in range(N_CORES):
        xt_c = np.ascontiguousarray(x_flat[c * ROWS_PC:(c + 1) * ROWS_PC].T[perm])
        m = {
            "attn": attn16_h,
            "xt": np.ascontiguousarray(xt_c[:S16]).astype(np.float16),
        }
        if PARTIAL_FP8:
            m["attn8"] = attn8_h
            m["xt8"] = np.ascontiguousarray(xt_c[S16:]).astype(NP_F8)
        in_maps_b.append(m)

    res_b = _run(nc_b, in_maps_b, **run_kwargs)
    LAST_RESULTS = (res_a, res_b)
    out = np.concatenate(
        [res_b.results[c]["out"] for c in range(N_CORES)], axis=0
    ).astype(np.float32)
    return out.reshape(B, D, OUT_F)


# revision 34
# speedup vs baseline: 1.1286x; 1.1286x over previous
"""Trainium2 Bass kernel for RandomSparseNewMlp.

Math (reference):
    attn = (einsum('ds,td->st', fc1_w, fc2_w) + fc2_b) * sparse_mask   # [1024, 1024]
    out  = gelu_erf(einsum('bds,st->bdt', x, attn))                    # [64, 768, 1024]

Strategy (8 cores, SPMD, two NEFF dispatches, no collectives):
  NEFF A ("attn"): the [1024,1024] attn matrix is 2D-sharded over the 8
    cores (4-way along s, 2-way along t) — each core computes one
    [256, 512] slice from its fc1/fc2^T column slices, applies bias
    (folded into the contraction as an extra K-row: ones row in fc1,
    bias row in fc2^T, K padded 4096 -> 4224 = 33*128) and the sparse
    mask, and returns the masked fp16 slice.  The host concatenates the
    8 slices (pure layout, no arithmetic).  All NEFF A operands are
    host-pre-rearranged to partition-major [128, ...] layout so every
    DMA descriptor is a multi-KB contiguous run (512 B descriptors made
    the issuing engines descriptor-generation-bound).  fc2 streams on
    the sync DMA queue while fc1 + mask stream in parallel on the act
    queue, kd-chunked so the kd=0 matmuls start after ~1.5 us.
  NEFF B ("mlp"): data-parallel shard of x over batch; core c handles
    rows [c*6144, (c+1)*6144) of the flattened [49152, 1024] x, computes
    gelu(x @ attn) with the gathered attn as a replicated input.
    x^T is loaded in 4 r-blocks of [128, k, 1536] (3 KB contiguous
    descriptors per partition row, double-buffered).  The first 4
    row-tiles run k-outer so each arriving (attn_k, x_k) chunk unlocks
    8 matmuls — the PE stays ahead of the DMA stream during the
    cold-start window.  Outputs are stored fp16 in batches of 4
    row-tiles (the host upcasts); the last group stores per-row-tile to
    shorten the drain tail.

  PARTIAL_FP8: the 256 lowest-energy attn rows (by exact row energy of
    the fp16 attn, a pure host-side calibration scan) are permuted to
    the tail k-chunks and contracted as ONE DoubleRow fp8 matmul pair
    (e4m3, scale 1, K=256 at 2 rows/cycle) accumulating into the same
    PSUM group as the six fp16 k-chunks.  Cuts the per-row-tile matmul
    time ~11% for a verified-by-emulation absmax/scale of ~1.7e-2
    (gate 2e-2).  The s-permutation is applied on the host between the
    two NEFFs (slicing only; contraction order is matmul-invariant).
"""

import numpy as np
import ml_dtypes
from contextlib import ExitStack

import concourse.bass as bass  # noqa: F401  (engine registration side effects)
import concourse.mybir as mybir
import concourse.tile as tile
from concourse import bacc
from concourse import bass_utils

PARTIAL_FP8 = True

P = 128
B, D = 64, 768
IN_F, HID_F, OUT_F = 1024, 4096, 1024
N_CORES = 8
ROWS = B * D                    # 49152
ROWS_PC = ROWS // N_CORES       # 6144
KH = HID_F + P                  # 4224 = 33*128 (hidden + bias/ones row, padded)
KD = KH // P                    # 33
S_TILES = IN_F // P             # 8
K16 = S_TILES - 2 if PARTIAL_FP8 else S_TILES   # fp16 k-chunks
S16 = K16 * P                   # fp16 s-rows (768 with fp8 on)
NB = 512                        # matmul moving free dim / PSUM bank
RB = 1536                       # xt r-block width per DMA batch
NRB = ROWS_PC // RB             # 4
RT_PER_RB = RB // P             # 12
STG = 4                         # rt per output-store batch / k-outer start group
S_SH, T_SH = 4, 2               # attn sharding grid: 4 along s, 2 along t
S_SL = IN_F // S_SH             # 256 rows of attn per core
T_SL = OUT_F // T_SH            # 512 cols of attn per core

F32 = mybir.dt.float32
F16 = mybir.dt.float16
F8 = mybir.dt.float8e4
NP_F8 = ml_dtypes.float8_e4m3


def _trace_attn_kernel(tc, aslice, fc1s, fc2ts, masks):
    """Per-core attn slice (all partition-major [128, ...] layouts):

    fc1s  [128, 33*256] fp16 : fc1 (K-extended) cols, (p, kd, s) order
    fc2ts [128, 33*512] fp16 : fc2^T (K-extended) cols, (p, kd, t) order
    masks [128, 2*512]  f32  : mask slice, (p, j, t) order
    aslice[128, 2*512]  f16  : output slice, (p, j, t) order
    """
    nc = tc.nc
    fc1_r = fc1s.rearrange("p (k s) -> p k s", k=KD)    # [128, 33, 256]
    fc2_r = fc2ts.rearrange("p (k t) -> p k t", k=KD)   # [128, 33, 512]
    mask_r = masks.rearrange("p (j t) -> p j t", j=2)   # [128, 2, 512]
    out_r = aslice.rearrange("p (j t) -> p j t", j=2)

    with ExitStack() as ctx:
        spool = ctx.enter_context(tc.tile_pool(name="spool", bufs=1))
        ppool = ctx.enter_context(tc.tile_pool(name="ppool", bufs=2, space="PSUM"))
        f2_sb = spool.tile([P, KD, T_SL], F16)
        f1_sb = spool.tile([P, KD, S_SL], F16)
        mask_sb = spool.tile([P, 2, T_SL], F16)
        out_sb = spool.tile([P, 2, T_SL], F16)
        # Dual-queue weight streaming, kd-chunked for pacing, alternating
        # fc2 (the big stream) across BOTH queues in kd order so the
        # matmul stream is never starved by a single queue's issue rate;
        # fc1 chunks interleave (smaller, ahead of need), mask last.
        CH = 3
        q = [nc.sync, nc.scalar]
        # tiny first chunk so the kd=0 matmul starts as early as possible;
        # taper the last chunks to single kd so the final matmuls aren't
        # gated on one multi-kd block.
        TAPER = 30
        nc.sync.dma_start(f2_sb[:, 0:1, :], fc2_r[:, 0:1, :])
        nc.scalar.dma_start(f1_sb[:, 0:1, :], fc1_r[:, 0:1, :])
        chunks = list(range(1, TAPER, CH)) + list(range(TAPER, KD))
        for i, c in enumerate(chunks):
            n = min(CH, TAPER - c) if c < TAPER else 1
            q[i % 2].dma_start(f2_sb[:, c:c + n, :], fc2_r[:, c:c + n, :])
            q[(i + 1) % 2].dma_start(f1_sb[:, c:c + n, :], fc1_r[:, c:c + n, :])
        nc.scalar.dma_start(mask_sb, mask_r)
        psums = [ppool.tile([P, NB], F32, name="ap") for _ in range(2)]
        for kd in range(KD):
            for si in range(2):
                nc.tensor.matmul(
                    psums[si],
                    f1_sb[:, kd, si * P:(si + 1) * P],
                    f2_sb[:, kd, :],
                    start=(kd == 0),
                    stop=(kd == KD - 1),
                )
        for si in range(2):
            nc.vector.tensor_mul(out_sb[:, si, :], psums[si], mask_sb[:, si, :])
            nc.sync.dma_start(out_r[:, si:si + 1, :], out_sb[:, si:si + 1, :])


def _trace_mlp_kernel(tc, out, attn, xt, attn8=None, xt8=None):
    """out[6144,1024] (fp16) = gelu(xT^T @ attn) for this core's row shard.

    attn [768|1024, 1024] f16, xt [768|1024, 6144] f16; with PARTIAL_FP8
    attn8 [256, 1024] fp8e4 and xt8 [256, 6144] fp8e4 carry the two tail
    k-chunks, contracted via one DoubleRow matmul per PSUM bank.
    """
    nc = tc.nc
    gelu = mybir.ActivationFunctionType.Gelu
    dr = mybir.MatmulPerfMode.DoubleRow
    attn_r = attn.rearrange("(k p) t -> p k t", p=P)    # [128, K16, 1024]
    xt_r = xt.rearrange("(k p) r -> p k r", p=P)        # [128, K16, 6144]
    if PARTIAL_FP8:
        attn8_r = attn8.rearrange("(k p) t -> p k t", p=P)   # [128, 2, 1024]
        xt8_r = xt8.rearrange("(k p) r -> p k r", p=P)       # [128, 2, 6144]

    with ExitStack() as ctx:
        consts = ctx.enter_context(tc.tile_pool(name="consts", bufs=1))
        attn_sb = consts.tile([P, K16, OUT_F], F16)
        attn8_sb = (consts.tile([P, 2, OUT_F], F8, name="attn8_sb")
                    if PARTIAL_FP8 else None)
        xpool = ctx.enter_context(tc.tile_pool(name="xpool", bufs=3))
        x8pool = ctx.enter_context(tc.tile_pool(name="x8pool", bufs=3)) \
            if PARTIAL_FP8 else None
        opool = ctx.enter_context(tc.tile_pool(name="opool", bufs=4))
        mpool = ctx.enter_context(tc.tile_pool(name="main_psum", bufs=8, space="PSUM"))

        # Interleave attn k-chunks with ONLY the first store-group's x
        # columns (W0) so the k-outer matmuls become runnable k-by-k with
        # a minimal DMA prefix; the rest of r-block 0 follows batched.
        W0 = STG * P
        xt0 = xpool.tile([P, K16, RB], F16, name="xt")
        xt8_0 = (x8pool.tile([P, 2, RB], F8, name="xt8")
                 if PARTIAL_FP8 else None)
        # Prefix split across BOTH HWDGE queues: attn k-chunks on sync,
        # the matching x columns on act — two parallel trigger streams
        # during the DMA ramp window.
        for k in range(K16):
            nc.sync.dma_start(attn_sb[:, k:k + 1, :], attn_r[:, k:k + 1, :])
            nc.scalar.dma_start(xt0[:, k, 0:W0], xt_r[:, k, 0:W0])
        if PARTIAL_FP8:
            nc.scalar.dma_start(xt8_0[:, :, 0:W0], xt8_r[:, :, 0:W0])
            nc.sync.dma_start(attn8_sb, attn8_r)
        nc.sync.dma_start(xt0[:, :, W0:RB], xt_r[:, :, W0:RB])
        if PARTIAL_FP8:
            nc.scalar.dma_start(xt8_0[:, :, W0:RB], xt8_r[:, :, W0:RB])

        def mm_rt(xtile, x8tile, c0, pa, pb, split=False):
            if split:
                # final row-tile: close pa's group ~1.5us before pb's so
                # act+store of the pa half overlap the pb matmul chain
                for bank, off in ((pa, 0), (pb, NB)):
                    for k in range(K16):
                        xs = xtile[:, k, c0:c0 + P]
                        last = (not PARTIAL_FP8) and (k == K16 - 1)
                        nc.tensor.matmul(bank, xs, attn_sb[:, k, off:off + NB],
                                         start=(k == 0), stop=last)
                    if PARTIAL_FP8:
                        x8s = x8tile[:, 0:2, c0:c0 + P]
                        nc.tensor.matmul(bank, x8s,
                                         attn8_sb[:, 0:2, off:off + NB],
                                         start=False, stop=True, perf_mode=dr)
                return
            for k in range(K16):
                xs = xtile[:, k, c0:c0 + P]
                last = (not PARTIAL_FP8) and (k == K16 - 1)
                nc.tensor.matmul(pa, xs, attn_sb[:, k, 0:NB],
                                 start=(k == 0), stop=last)
                nc.tensor.matmul(pb, xs, attn_sb[:, k, NB:OUT_F],
                                 start=(k == 0), stop=last)
            if PARTIAL_FP8:
                x8s = x8tile[:, 0:2, c0:c0 + P]
                nc.tensor.matmul(pa, x8s, attn8_sb[:, 0:2, 0:NB],
                                 start=False, stop=True, perf_mode=dr)
                nc.tensor.matmul(pb, x8s, attn8_sb[:, 0:2, NB:OUT_F],
                                 start=False, stop=True, perf_mode=dr)

        def evict(ot, j, pa, pb, r0, batched):
            if not batched:
                # drain path: store each half right after its activation
                dst = out[r0:r0 + P, :].rearrange("(g p) t -> p g t", p=P)
                nc.scalar.activation(ot[:, j, 0:NB], pa, gelu)
                nc.sync.dma_start(dst[:, :, 0:NB], ot[:, j:j + 1, 0:NB])
                nc.scalar.activation(ot[:, j, NB:OUT_F], pb, gelu)
                nc.sync.dma_start(dst[:, :, NB:OUT_F], ot[:, j:j + 1, NB:OUT_F])
                return
            nc.scalar.activation(ot[:, j, 0:NB], pa, gelu)
            nc.scalar.activation(ot[:, j, NB:OUT_F], pb, gelu)
            if j == STG - 1:
                nc.sync.dma_start(
                    out[r0 - (STG - 1) * P:r0 + P, :].rearrange(
                        "(g p) t -> p g t", p=P),
                    ot,
                )

        for rb in range(NRB):
            if rb == 0:
                xtile, x8tile = xt0, (xt8_0 if PARTIAL_FP8 else None)
            else:
                xtile = xpool.tile([P, K16, RB], F16, name="xt")
                nc.sync.dma_start(xtile, xt_r[:, :, rb * RB:(rb + 1) * RB])
                if PARTIAL_FP8:
                    x8tile = x8pool.tile([P, 2, RB], F8, name="xt8")
                    nc.sync.dma_start(x8tile, xt8_r[:, :, rb * RB:(rb + 1) * RB])
                else:
                    x8tile = None
            if rb == 0:
                # k-outer start group: 4 rt x 2 banks per k-chunk.
                ps = [mpool.tile([P, NB], F32, name="mp") for _ in range(2 * STG)]
                for k in range(K16):
                    for j in range(STG):
                        xs = xtile[:, k, j * P:(j + 1) * P]
                        last = (not PARTIAL_FP8) and (k == K16 - 1)
                        nc.tensor.matmul(ps[2 * j], xs, attn_sb[:, k, 0:NB],
                                         start=(k == 0), stop=last)
                        nc.tensor.matmul(ps[2 * j + 1], xs, attn_sb[:, k, NB:OUT_F],
                                         start=(k == 0), stop=last)
                if PARTIAL_FP8:
                    for j in range(STG):
                        x8s = x8tile[:, 0:2, j * P:(j + 1) * P]
                        nc.tensor.matmul(ps[2 * j], x8s, attn8_sb[:, 0:2, 0:NB],
                                         start=False, stop=True, perf_mode=dr)
                        nc.tensor.matmul(ps[2 * j + 1], x8s,
                                         attn8_sb[:, 0:2, NB:OUT_F],
                                         start=False, stop=True, perf_mode=dr)
                ot = opool.tile([P, STG, OUT_F], F16, name="ot")
                for j in range(STG):
                    evict(ot, j, ps[2 * j], ps[2 * j + 1], (STG - 1) * P, True)
                rt_start = STG
            else:
                rt_start = 0
            for rt in range(rt_start, RT_PER_RB):
                c0 = rt * P
                last_group = (rb == NRB - 1 and rt >= RT_PER_RB - STG)
                if rt % STG == 0 and not last_group:
                    ot = opool.tile([P, STG, OUT_F], F16, name="ot")
                elif last_group:
                    ot = opool.tile([P, 1, OUT_F], F16, name="ot")
                pa = mpool.tile([P, NB], F32, name="mp")
                pb = mpool.tile([P, NB], F32, name="mp")
                final_rt = (rb == NRB - 1 and rt == RT_PER_RB - 1)
                mm_rt(xtile, x8tile, c0, pa, pb, split=final_rt)
                if last_group:
                    evict(ot, 0, pa, pb, rb * RB + c0, False)
                else:
                    evict(ot, rt % STG, pa, pb, rb * RB + c0, True)


_NC_CACHE = {}
LAST_RESULTS = None


def _build_attn():
    if "attn" in _NC_CACHE:
        return _NC_CACHE["attn"]
    nc = bacc.Bacc("TRN2", target_bir_lowering=False, debug=False,
                   num_devices=N_CORES)
    fc1s = nc.dram_tensor("fc1s", [P, KD * S_SL], F16, kind="ExternalInput").ap()
    fc2ts = nc.dram_tensor("fc2ts", [P, KD * T_SL], F16, kind="ExternalInput").ap()
    masks = nc.dram_tensor("masks", [P, 2 * T_SL], F16, kind="ExternalInput").ap()
    aslice = nc.dram_tensor("aslice", [P, 2 * T_SL], F16,
                            kind="ExternalOutput").ap()
    with tile.TileContext(nc) as tc:
        _trace_attn_kernel(tc, aslice, fc1s, fc2ts, masks)
    nc.compile()
    _NC_CACHE["attn"] = nc
    return nc


def _build_mlp():
    if "mlp" in _NC_CACHE:
        return _NC_CACHE["mlp"]
    nc = bacc.Bacc("TRN2", target_bir_lowering=False, debug=False,
                   num_devices=N_CORES)
    attn = nc.dram_tensor("attn", [S16, OUT_F], F16, kind="ExternalInput").ap()
    xt = nc.dram_tensor("xt", [S16, ROWS_PC], F16, kind="ExternalInput").ap()
    attn8 = xt8 = None
    if PARTIAL_FP8:
        attn8 = nc.dram_tensor("attn8", [2 * P, OUT_F], F8,
                               kind="ExternalInput").ap()
        xt8 = nc.dram_tensor("xt8", [2 * P, ROWS_PC], F8,
                             kind="ExternalInput").ap()
    out = nc.dram_tensor("out", [ROWS_PC, OUT_F], F16, kind="ExternalOutput").ap()
    with tile.TileContext(nc) as tc:
        _trace_mlp_kernel(tc, out, attn, xt, attn8, xt8)
    nc.compile()
    _NC_CACHE["mlp"] = nc
    return nc


def _run(nc, in_maps, **kwargs):
    return bass_utils.run_bass_kernel_spmd(
        nc, in_maps, core_ids=list(range(N_CORES)), **kwargs
    )


def _pmajor(a, inner):
    """[KD*128, inner] row-major -> [128, KD*inner] partition-major."""
    kd = a.shape[0] // P
    return np.ascontiguousarray(
        a.reshape(kd, P, inner).transpose(1, 0, 2).reshape(P, kd * inner)
    )


def kernel(x, fc1_w, fc2_w, fc2_b, sparse_mask, **run_kwargs):
    global LAST_RESULTS
    nc_a = _build_attn()
    nc_b = _build_mlp()

    # --- host prep: K-extended fp16 weight slices (layout only) ---
    fc1e = np.concatenate(
        [
            np.asarray(fc1_w, np.float32),
            np.ones((1, IN_F), np.float32),
            np.zeros((P - 1, IN_F), np.float32),
        ],
        axis=0,
    ).astype(np.float16)
    fc2te = np.concatenate(
        [
            np.asarray(fc2_w, np.float32).T,
            np.asarray(fc2_b, np.float32)[None, :],
            np.zeros((P - 1, OUT_F), np.float32),
        ],
        axis=0,
    ).astype(np.float16)
    mask = np.asarray(sparse_mask, np.float32)

    in_maps_a = []
    for c in range(N_CORES):
        si, tj = divmod(c, T_SH)
        in_maps_a.append({
            "fc1s": _pmajor(fc1e[:, si * S_SL:(si + 1) * S_SL], S_SL),
            "fc2ts": _pmajor(fc2te[:, tj * T_SL:(tj + 1) * T_SL], T_SL),
            "masks": _pmajor(
                np.ascontiguousarray(
                    mask[si * S_SL:(si + 1) * S_SL, tj * T_SL:(tj + 1) * T_SL]
                ).astype(np.float16), T_SL),
        })

    res_a = _run(nc_a, in_maps_a, **run_kwargs)

    # --- host gather of attn slices (pure concatenation / relayout) ---
    attn_full = np.empty((IN_F, OUT_F), np.float16)
    for c in range(N_CORES):
        si, tj = divmod(c, T_SH)
        attn_full[si * S_SL:(si + 1) * S_SL, tj * T_SL:(tj + 1) * T_SL] = (
            res_a.results[c]["aslice"].reshape(P, 2, T_SL)
            .transpose(1, 0, 2).reshape(S_SL, T_SL)
        )

    if PARTIAL_FP8:
        # Quantization calibration: permute the 256 lowest-energy attn
        # rows (and the matching x columns) into the fp8 tail chunks.
        energy = (attn_full.astype(np.float32) ** 2).sum(axis=1)
        perm = np.argsort(-energy)
        attn_p = attn_full[perm]
        attn16_h = np.ascontiguousarray(attn_p[:S16])
        attn8_h = np.ascontiguousarray(attn_p[S16:]).astype(NP_F8)
    else:
        perm = np.arange(IN_F)
        attn16_h = attn_full
        attn8_h = None

    x_flat = np.asarray(x, np.float32).reshape(ROWS, IN_F)
    in_maps_b = []
    for c in range(N_CORES):
        xt_c = np.ascontiguousarray(x_flat[c * ROWS_PC:(c + 1) * ROWS_PC].T[perm])
        m = {
            "attn": attn16_h,
            "xt": np.ascontiguousarray(xt_c[:S16]).astype(np.float16),
        }
        if PARTIAL_FP8:
            m["attn8"] = attn8_h
            m["xt8"] = np.ascontiguousarray(xt_c[S16:]).astype(NP_F8)
        in_maps_b.append(m)

    res_b = _run(nc_b, in_maps_b, **run_kwargs)
    LAST_RESULTS = (res_a, res_b)
    out = np.concatenate(
        [res_b.results[c]["out"] for c in range(N_CORES)], axis=0
    ).astype(np.float32)
    return out.reshape(B, D, OUT_F)
